# revision 22
# baseline (speedup 1.0000x reference)
"""Trainium2 Bass kernel for nn_L2GTraversal (leaf->level1->root point-cloud net).

Strategy (8 NeuronCores, data-parallel over leaves):
  - 64 leaves x 2048 points; core m owns leaves 8m..8m+7 (16384 points).
  - All activations kept TRANSPOSED (channels on partitions, points on the
    free dim) so every layer is lhsT=weight (stationary), rhs=activation^T,
    and the per-leaf max-pool is a free-dim reduce.
  - Algebraic fold: proj@We1[3:] with proj = relu1@Wp2 + bp2 is folded to
    relu1@(Wp2@We1[3:]) + const-bias, removing one 128x128 GEMM per point.
  - relu/max/bias commute: the last-layer relu+bias is applied after the
    per-leaf max-pool (on 512 values/leaf instead of 2048x512).
  - All matmul operands are bf16 (weights host-rounded, activations
    engine-rounded); bf16 stationary weights get 2x-faster LDWEIGHTS via
    fast-weight-load, and the PE runs 1 cycle/row (same as fp32r).
  - Relative coords (pts - leaf_center) are computed on the HOST and DMA'd
    in as relT (3, PTS) — computing them on GpSimd costs 29us/leaf and
    serializes the whole pipeline.
  - Chunk-pipelined emission (mm1 issued two chunks ahead) keeps Tensor/
    Scalar/Vector continuously busy so the HAM clock stays at 2.4 GHz.
    pe2 accumulates chunk PAIRS into (128,1024) two-bank PSUM tiles so the
    Vector engine drains them with half as many (fixed-overhead-dominated)
    reduce passes.
  - Level-1 aggregation is core-local (leaves 8m..8m+7 are exactly parent
    m's children).  The root needs the cross-core max of the per-parent
    relu(Wa1@[level1_m; relpos]) vectors: one tiny AllGather of (128,4)
    bf16 (~10us cheaper than AllReduce here), then each core reduces the
    gathered blocks locally and computes the root row.
  - A dummy AllGather issued at kernel start absorbs the collective
    stream-entry cost (~11.5us trigger latency) concurrently with the
    leaf pipeline, so the real tail AllGather triggers in ~1us.
  - Level1 is folded OUT of the root's critical path: g2 = relu(WaF^T m1
    + bg2) with WaF = Wa2@Wa1[:512] and bg2 = ba1 + ba2@Wa1[:512] +
    r2@Wa1[512:] host-precomputed (bg2 per-core).  The level1 output rows
    are computed during the collective window instead of before it.

Host side does only: index gathers, transposes/slicing for the chosen
sharding layout, the one-time weight fold, and output reassembly.
"""

import os

import numpy as np

import concourse.bass as bass  # noqa: F401
import concourse.mybir as mybir
import concourse.tile as tile
from concourse import bacc
from concourse.bass_utils import run_bass_kernel_spmd

NCORES = 8
L, K, C = 64, 2048, 32
LPC = L // NCORES            # leaves per core
PTS = LPC * K                # points per core
D_PROJ, D_HID, D = 128, 256, 512
CH = 512                     # point-chunk (matmul free dim)
CPL = K // CH                # chunks per leaf
F32 = mybir.dt.float32
BF16 = mybir.dt.bfloat16


def _bf16(a):
    import ml_dtypes
    return np.ascontiguousarray(np.asarray(a, np.float32).astype(
        ml_dtypes.bfloat16))


def _emit(tc, tin, tout):
    nc = tc.nc
    import contextlib

    ctx = contextlib.ExitStack()
    with ctx:
        const = ctx.enter_context(tc.tile_pool(name="const", bufs=1))
        act = ctx.enter_context(tc.tile_pool(name="act", bufs=1))
        red = ctx.enter_context(tc.tile_pool(name="red", bufs=1))
        agg = ctx.enter_context(tc.tile_pool(name="agg", bufs=1))
        psp = ctx.enter_context(tc.tile_pool(name="psum", bufs=1, space="PSUM"))
        dram = ctx.enter_context(tc.tile_pool(name="dram", bufs=1, space="DRAM"))

        def cload(name, shape, dt=F32):
            t = const.tile(list(shape), dt, name=name, tag=name)
            nc.sync.dma_start(out=t, in_=tin[name][:, :])
            return t

        RELU = mybir.ActivationFunctionType.Relu

        # ---- DMA priority order: first weights + leaf-0 data the pipeline
        # start needs, then the rest of the leaves, aggregation weights last --
        wp1 = cload("wp1", (32, 128), BF16)
        bp1 = cload("bp1", (128, 1))

        pfT, relT = {}, {}

        def load_leaf(l, part=None):
            # part: (tag_suffix, lo, hi) chunk range, else whole leaf
            lo, hi = (0, K) if part is None else part
            sfx = "" if part is None else f"_{lo}"
            t = const.tile([32, hi - lo], BF16, name=f"pfT{l}{sfx}",
                           tag=f"pfT{l}{sfx}")
            nc.sync.dma_start(out=t, in_=tin["featsT"][:, l * K + lo:l * K + hi])
            r = const.tile([3, hi - lo], BF16, name=f"relT{l}{sfx}",
                           tag=f"relT{l}{sfx}")
            nc.sync.dma_start(out=r, in_=tin["relT"][:, l * K + lo:l * K + hi])
            for c in range((hi - lo) // CH):
                pfT[l * CPL + lo // CH + c] = (t, c)
                relT[l * CPL + lo // CH + c] = (r, c)

        load_leaf(0, (0, 2 * CH))
        w2e = cload("w2e", (128, 256), BF16)
        we1a = cload("we1a", (3, 256), BF16)
        be1f = cload("be1f", (128, 2))
        we2 = []
        for kt in range(2):
            t = const.tile([128, 512], BF16, name=f"we2_{kt}", tag=f"we2_{kt}")
            nc.sync.dma_start(out=t, in_=tin["we2"][kt * 128:(kt + 1) * 128, :])
            we2.append(t)
        load_leaf(0, (2 * CH, K))
        be2c = cload("be2c", (128, 4))
        for l in range(1, LPC):
            load_leaf(l)

        def pf_sl(q):
            t, c = pfT[q]
            return t[:, c * CH:(c + 1) * CH]

        def rel_sl(q):
            t, c = relT[q]
            return t[:, c * CH:(c + 1) * CH]

        # aggregation weights (needed only at the end)
        wa1 = []
        for kt in range(4):
            t = const.tile([128, 512], BF16, name=f"wa1_{kt}", tag=f"wa1_{kt}")
            nc.sync.dma_start(out=t, in_=tin["wa1"][kt * 128:(kt + 1) * 128, :])
            wa1.append(t)
        wa1r = cload("wa1r", (3, 512), BF16)
        ba1c = cload("ba1c", (128, 4))
        wa2 = []
        for kt in range(4):
            t = const.tile([128, 512], BF16, name=f"wa2_{kt}", tag=f"wa2_{kt}")
            nc.sync.dma_start(out=t, in_=tin["wa2"][kt * 128:(kt + 1) * 128, :])
            wa2.append(t)
        ba2c = cload("ba2c", (128, 4))
        ba2r = cload("ba2r", (1, 512))
        relcT = cload("relcT", (3, LPC), BF16)
        waf = []
        for kt in range(4):
            t = const.tile([128, 512], BF16, name=f"waf_{kt}", tag=f"waf_{kt}")
            nc.sync.dma_start(out=t, in_=tin["waf"][kt * 128:(kt + 1) * 128, :])
            waf.append(t)
        bg2c = cload("bg2c", (128, 4))

        # leaf features (relu+bias applied), one column per leaf; bf16 so
        # they can be the moving operand of the bf16 aggregation matmuls
        lfv = [const.tile([128, LPC], BF16, name=f"lfv{o2}", tag=f"lfv{o2}")
               for o2 in range(4)]

        # warmup collective: pays the collective-stream entry costs while
        # the leaf pipeline runs, so the real AllGather at the tail doesn't
        if os.environ.get("KSKIP_CC") != "1":
            wsb = agg.tile([128, 4], BF16, name="wsb", tag="wsb")
            nc.vector.memset(wsb, 0.0)
            win = dram.tile([128, 4], BF16, name="wrmin", tag="wrmin")
            wout = dram.tile([NCORES, 128, 4], BF16, name="wrmout",
                             tag="wrmout")
            nc.gpsimd.dma_start(out=win, in_=wsb)
            nc.gpsimd.collective_compute(
                "AllGather",
                mybir.AluOpType.bypass,
                replica_groups=[list(range(NCORES))],
                ins=[win.opt()],
                outs=[wout.opt()],
            )

        relu1 = {}   # chunk -> (128, CH) sbuf tile
        hTd = {}     # pair -> [ot0, ot1] (128, 2*CH) bf16 tiles (chunk pair)
        mx = {}      # (leaf, o2) -> (128, 2) per-pair max columns

        def emit_mm1(q):
            l, c = q // CPL, q % CPL
            ps1 = psp.tile([128, CH], F32, name=f"ps1_q{q}", tag="ps1", bufs=2)
            nc.tensor.matmul(ps1, wp1, pf_sl(q), start=True, stop=True)
            t = act.tile([128, CH], BF16, name=f"relu1_q{q}", tag="relu1",
                         bufs=4)
            nc.scalar.activation(t, ps1, RELU, bias=bp1[:, 0:1])
            relu1[q] = t

        def emit_pe1(q):
            l, c = q // CPL, q % CPL
            if c % 2 == 0:
                hTd[q // 2] = [
                    act.tile([128, 2 * CH], BF16, name=f"hTd_p{q // 2}o{ot}",
                             tag=f"hTd{ot}", bufs=3)
                    for ot in range(2)]
            half = slice((c % 2) * CH, (c % 2 + 1) * CH)
            for ot in range(2):
                sl = slice(ot * 128, (ot + 1) * 128)
                pe1 = psp.tile([128, CH], F32, name=f"pe1_q{q}o{ot}",
                               tag="pe1", bufs=2)
                nc.tensor.matmul(pe1, w2e[:, sl], relu1[q],
                                 start=True, stop=False)
                nc.tensor.matmul(pe1, we1a[:, sl], rel_sl(q),
                                 start=False, stop=True)
                nc.scalar.activation(hTd[q // 2][ot][:, half], pe1, RELU,
                                     bias=be1f[:, ot:ot + 1])
            del relu1[q]

        NQ = LPC * CPL

        def pe2_half(pe2, o2, hd, h):
            sl = slice(o2 * 128, (o2 + 1) * 128)
            half = slice(h * CH, (h + 1) * CH)
            nc.tensor.matmul(pe2[:, half], we2[0][:, sl],
                             hd[0][:, half], start=True, stop=False)
            nc.tensor.matmul(pe2[:, half], we2[1][:, sl],
                             hd[1][:, half], start=False, stop=True)

        def emit_rest(q):
            l, c = q // CPL, q % CPL
            emit_pe1(q)
            # lookahead mm1 AFTER pe1 so this chunk's hTd activations sit
            # ahead of relu1(q+2) in the scalar queue
            if q + 2 < NQ:
                emit_mm1(q + 2)
            if c % 2 == 0:
                return
            # chunk pair (q-1, q): per o2-pair, run both half-0 GEMMs (data
            # ready since last iteration) before any half-1 GEMM needs the
            # hTd activations of chunk q — hides the scalar latency
            hd = hTd[q // 2]
            for ob in (0, 2):
                pe2 = {}
                for o2 in (ob, ob + 1):
                    pe2[o2] = psp.tile([128, 2 * CH], F32,
                                       name=f"pe2_q{q}o{o2}", tag="pe2",
                                       bufs=2)
                    pe2_half(pe2[o2], o2, hd, 0)
                for o2 in (ob, ob + 1):
                    pe2_half(pe2[o2], o2, hd, 1)
                    if c == 1:
                        mx[(l, o2)] = red.tile(
                            [128, 2], F32, name=f"mx_l{l}o{o2}",
                            tag=f"mx{o2}", bufs=2)
                    nc.vector.reduce_max(
                        out=mx[(l, o2)][:, c // 2:c // 2 + 1],
                        in_=pe2[o2], axis=mybir.AxisListType.X)
            del hTd[q // 2]
            if c == CPL - 1:
                for o2 in range(4):
                    lfm = red.tile([128, 1], F32, name=f"lfm_l{l}o{o2}",
                                   tag=f"lfm{o2}", bufs=2)
                    nc.vector.reduce_max(out=lfm, in_=mx.pop((l, o2)),
                                         axis=mybir.AxisListType.X)
                    nc.scalar.activation(lfv[o2][:, l:l + 1], lfm, RELU,
                                         bias=be2c[:, o2:o2 + 1])

        # ---- software-pipelined emission of the per-point encoder ----
        emit_mm1(0)
        emit_mm1(1)
        for q in range(NQ):
            emit_rest(q)

        # ---- leaf features out (convert to f32 for the output DMA) ----
        for o2 in range(4):
            cvt = agg.tile([128, LPC], F32, name=f"lfvf{o2}", tag=f"lfvf{o2}")
            nc.vector.tensor_copy(cvt, lfv[o2])
            nc.sync.dma_start(out=tout[o2 * 128:(o2 + 1) * 128, 0:LPC],
                              in_=cvt)

        # ---- level 1: g1 = relu(Wa1^T [lfv; relc] + ba1); max; @Wa2 + ba2 ----
        m1 = []
        for o2 in range(4):
            sl = slice(o2 * 128, (o2 + 1) * 128)
            psA = psp.tile([128, LPC], F32, name=f"psA{o2}", tag="pe2", bufs=2)
            for kt in range(4):
                nc.tensor.matmul(psA, wa1[kt][:, sl], lfv[kt],
                                 start=(kt == 0), stop=False)
            nc.tensor.matmul(psA, wa1r[:, sl], relcT, start=False, stop=True)
            g1 = agg.tile([128, LPC], BF16, name=f"g1_{o2}", tag=f"g1_{o2}")
            nc.scalar.activation(g1, psA, RELU, bias=ba1c[:, o2:o2 + 1])
            m = agg.tile([128, 1], BF16, name=f"m1_{o2}", tag=f"m1_{o2}")
            nc.vector.reduce_max(out=m, in_=g1, axis=mybir.AxisListType.X)
            m1.append(m)

        # ---- root pre-activation, with level1 folded out of the critical
        # path: Wa1^T(Wa2^T m1 + ba2) + Wa1r^T r2 + ba1 = WaF^T m1 + bg2
        # (WaF = Wa2 @ Wa1[:512] and bg2 are host-precomputed, bg2 per-core)
        g2 = agg.tile([128, 4], BF16, name="g2", tag="g2")
        for o2 in range(4):
            sl = slice(o2 * 128, (o2 + 1) * 128)
            psR = psp.tile([128, 1], F32, name=f"psR{o2}", tag="ps1", bufs=2)
            for kt in range(4):
                nc.tensor.matmul(psR, waf[kt][:, sl], m1[kt],
                                 start=(kt == 0), stop=(kt == 3))
            nc.scalar.activation(g2[:, o2:o2 + 1], psR, RELU,
                                 bias=bg2c[:, o2:o2 + 1])

        gm = agg.tile([128, 4], BF16, name="gm", tag="gm")
        if os.environ.get("KSKIP_CC") == "1":
            nc.vector.tensor_copy(gm, g2)
        else:
            crin = dram.tile([128, 4], BF16, name="crin", tag="crin")
            crout = dram.tile([NCORES, 128, 4], BF16, name="crout",
                              tag="crout")
            nc.sync.dma_start(out=crin, in_=g2)
            nc.gpsimd.collective_compute(
                "AllGather",
                mybir.AluOpType.bypass,
                replica_groups=[list(range(NCORES))],
                ins=[crin.opt()],
                outs=[crout.opt()],
            )
            gall = agg.tile([128, NCORES, 4], BF16, name="gall", tag="gall")
            nc.sync.dma_start(out=gall,
                               in_=crout[:, :, :].transpose([1, 0, 2]))

        # level1 output rows — computed during the collective window
        for o2 in range(4):
            sl = slice(o2 * 128, (o2 + 1) * 128)
            psL = psp.tile([128, 1], F32, name=f"psL{o2}", tag="ps1", bufs=2)
            for kt in range(4):
                nc.tensor.matmul(psL, wa2[kt][:, sl], m1[kt],
                                 start=(kt == 0), stop=(kt == 3))
            vf = agg.tile([128, 1], F32, name=f"lvl1f_{o2}", tag=f"lvl1f_{o2}")
            nc.scalar.add(vf, psL, ba2c[:, o2:o2 + 1])
            nc.sync.dma_start(out=tout[sl, LPC:LPC + 1], in_=vf)
            gall = gall.flatten_outer_dims() if False else gall
            # per-channel max over the 8 gathered blocks (stride-4 views)
            for c2 in range(4):
                nc.vector.reduce_max(out=gm[:, c2:c2 + 1],
                                     in_=gall[:, :, c2],
                                     axis=mybir.AxisListType.X)

        # root row as a single (1,512) vector: gm columns are the stationary
        # operand, Wa2 streams — 4 wide matmuls instead of 16 tiny ones
        psR2 = psp.tile([1, 512], F32, name="psR2", tag="ps1", bufs=2)
        for kt in range(4):
            nc.tensor.matmul(psR2, gm[:, kt:kt + 1], wa2[kt],
                             start=(kt == 0), stop=(kt == 3))
        rv = agg.tile([1, 512], F32, name="rootv", tag="rootv")
        nc.vector.tensor_tensor(out=rv, in0=psR2, in1=ba2r,
                                op=mybir.AluOpType.add)
        nc.sync.dma_start(out=tout[:, LPC + 1:LPC + 2], in_=rv)


_CACHE = {}


def _build():
    if "nc" in _CACHE:
        return _CACHE["nc"]
    nc = bacc.Bacc("TRN2", target_bir_lowering=False, debug=False,
                   num_devices=NCORES)
    shapes = {
        "featsT": ((32, PTS), BF16), "relT": ((3, PTS), BF16),
        "relcT": ((3, LPC), BF16), "waf": ((512, 512), BF16),
        "bg2c": ((128, 4), F32),
        "wp1": ((32, 128), BF16), "bp1": ((128, 1), F32),
        "w2e": ((128, 256), BF16), "we1a": ((3, 256), BF16),
        "be1f": ((128, 2), F32), "we2": ((256, 512), BF16),
        "be2c": ((128, 4), F32), "wa1": ((512, 512), BF16),
        "wa1r": ((3, 512), BF16), "ba1c": ((128, 4), F32),
        "wa2": ((512, 512), BF16), "ba2c": ((128, 4), F32),
        "ba2r": ((1, 512), F32),
    }
    tin = {name: nc.dram_tensor(name, list(shape), dt,
                                kind="ExternalInput").ap()
           for name, (shape, dt) in shapes.items()}
    tout = nc.dram_tensor("out", [512, LPC + 2], F32, kind="ExternalOutput").ap()
    with tile.TileContext(nc) as tc:
        _emit(tc, tin, tout)
    nc.compile()
    _CACHE["nc"] = nc
    return nc


def _prep_in_maps(inputs):
    f32 = np.float32
    coords = np.asarray(inputs["coords"], f32)
    feats = np.asarray(inputs["feats"], f32)
    leaf_indices = np.asarray(inputs["leaf_indices"])
    leaf_center_idx = np.asarray(inputs["leaf_center_idx"])
    l1_center_idx = np.asarray(inputs["l1_center_idx"])
    root_center_idx = int(np.asarray(inputs["root_center_idx"]))

    pts = coords[leaf_indices]            # (L, K, 3)
    pf = feats[leaf_indices]              # (L, K, C)
    centers = coords[leaf_center_idx]     # (L, 3)
    pp = coords[l1_center_idx]            # (B1, 3)
    rootc = coords[root_center_idx]       # (3,)

    Wp1 = np.asarray(inputs["Wp1"], f32)
    bp1 = np.asarray(inputs["bp1"], f32)
    Wp2 = np.asarray(inputs["Wp2"], f32)
    bp2 = np.asarray(inputs["bp2"], f32)
    We1 = np.asarray(inputs["We1"], f32)
    be1 = np.asarray(inputs["be1"], f32)
    We2 = np.asarray(inputs["We2"], f32)
    be2 = np.asarray(inputs["be2"], f32)
    Wa1 = np.asarray(inputs["Wa1"], f32)
    ba1 = np.asarray(inputs["ba1"], f32)
    Wa2 = np.asarray(inputs["Wa2"], f32)
    ba2 = np.asarray(inputs["ba2"], f32)

    # fold proj's second linear into the encoder first layer (fp64 for safety)
    We1a = np.ascontiguousarray(We1[0:3])                       # (3, 256)
    We1b = We1[3:131]                                           # (128, 256)
    W2e = (Wp2.astype(np.float64) @ We1b.astype(np.float64)).astype(f32)
    be1f = (be1.astype(np.float64)
            + bp2.astype(np.float64) @ We1b.astype(np.float64)).astype(f32)

    common = {
        "wp1": _bf16(Wp1),
        "bp1": np.ascontiguousarray(bp1.reshape(128, 1)),
        "w2e": _bf16(W2e),
        "we1a": _bf16(We1a),
        "be1f": np.ascontiguousarray(be1f.reshape(2, 128).T),
        "we2": _bf16(We2),
        "be2c": np.ascontiguousarray(be2.reshape(4, 128).T),
        "wa1": _bf16(Wa1[0:512]),
        "wa1r": _bf16(Wa1[512:515]),
        "ba1c": np.ascontiguousarray(ba1.reshape(4, 128).T),
        "wa2": _bf16(Wa2),
        "ba2c": np.ascontiguousarray(ba2.reshape(4, 128).T),
        "ba2r": np.ascontiguousarray(ba2.reshape(1, 512)),
        "waf": _bf16(Wa2.astype(np.float64) @ Wa1[0:512].astype(np.float64)),
    }

    in_maps = []
    for m in range(NCORES):
        sl = slice(m * LPC, (m + 1) * LPC)
        im = dict(common)
        im["featsT"] = _bf16(pf[sl].reshape(PTS, C).T)          # (32, PTS)
        rel = pts[sl] - centers[sl][:, None, :]                 # (LPC, K, 3)
        im["relT"] = _bf16(rel.reshape(PTS, 3).T)               # (3, PTS)
        im["relcT"] = _bf16((centers[sl] - pp[m]).T)
        r2 = (pp[m] - rootc).astype(np.float64)
        bg2 = (ba1.astype(np.float64)
               + ba2.astype(np.float64) @ Wa1[0:512].astype(np.float64)
               + r2 @ Wa1[512:515].astype(np.float64)).astype(f32)
        im["bg2c"] = np.ascontiguousarray(bg2.reshape(4, 128).T)
        in_maps.append(im)
    return in_maps


def _run(inputs, **kwargs):
    nc = _build()
    in_maps = _prep_in_maps(inputs)
    res = run_bass_kernel_spmd(nc, in_maps, core_ids=list(range(NCORES)),
                               **kwargs)
    out = np.empty((1 + NCORES + L, D), np.float32)
    out[0] = res.results[0]["out"][:, LPC + 1]
    for m in range(NCORES):
        out[1 + m] = res.results[m]["out"][:, LPC]
        out[1 + NCORES + m * LPC:1 + NCORES + (m + 1) * LPC] = \
            res.results[m]["out"][:, 0:LPC].T
    return out, res


def kernel(**inputs) -> np.ndarray:
    out, _ = _run(inputs)
    return out


# ---------------------------------------------------------------------------
# dev-only timing helpers (not used by kernel()); safe to keep — they only
# run when called explicitly from test.py.
# ---------------------------------------------------------------------------

def _pjrt_loop(nc, in_maps, iters):
    import time

    import jax
    from jax.experimental.shard_map import shard_map
    from jax.sharding import Mesh, NamedSharding, PartitionSpec

    from concourse.bass2jax import (_bass_exec_p, install_neuronx_cc_hook,
                                    partition_id_tensor)

    install_neuronx_cc_hook()
    pname = nc.partition_id_tensor.name if nc.partition_id_tensor else None
    in_names, out_names, out_avals, zero_outs = [], [], [], []
    for alloc in nc.m.functions[0].allocations:
        if not isinstance(alloc, mybir.MemoryLocationSet):
            continue
        name = alloc.memorylocations[0].name
        if alloc.kind == "ExternalInput":
            if name != pname:
                in_names.append(name)
        elif alloc.kind == "ExternalOutput":
            out_names.append(name)
            shape = tuple(alloc.tensor_shape)
            dtype = mybir.dt.np(alloc.dtype)
            out_avals.append(jax.core.ShapedArray(shape, dtype))
            zero_outs.append(np.zeros(shape, dtype))
    n_params = len(in_names)
    all_names = in_names + out_names
    if pname is not None:
        all_names = all_names + [pname]

    def _body(*args):
        operands = list(args)
        if pname is not None:
            operands.append(partition_id_tensor())
        outs = _bass_exec_p.bind(
            *operands, out_avals=tuple(out_avals), in_names=tuple(all_names),
            out_names=tuple(out_names), lowering_input_output_aliases=(),
            sim_require_finite=True, sim_require_nnan=True, nc=nc)
        return tuple(outs)

    ncores = len(in_maps)
    devices = jax.devices()[:ncores]
    mesh = Mesh(np.asarray(devices), ("core",))
    spec = PartitionSpec("core")
    donate = tuple(range(n_params, n_params + len(out_names)))
    fn = jax.jit(
        shard_map(_body, mesh=mesh,
                  in_specs=(spec,) * (n_params + len(out_names)),
                  out_specs=(spec,) * len(out_names), check_rep=False),
        donate_argnums=donate, keep_unused=True)
    sh = NamedSharding(mesh, spec)
    ins = [jax.device_put(
        np.concatenate([np.asarray(m[n]) for m in in_maps], axis=0), sh)
        for n in in_names]
    zs_proto = [np.zeros((ncores * z.shape[0], *z.shape[1:]), z.dtype)
                for z in zero_outs]
    outs = fn(*ins, *[jax.device_put(z, sh) for z in zs_proto])
    jax.block_until_ready(outs)
    times = []
    for _ in range(iters):
        zs = [jax.device_put(z, sh) for z in zs_proto]
        jax.block_until_ready(zs)
        t0 = time.perf_counter()
        outs = fn(*ins, *zs)
        jax.block_until_ready(outs)
        times.append(time.perf_counter() - t0)
    return times


def _time_hw(inputs, iters=20):
    nc = _build()
    in_maps = _prep_in_maps(inputs)
    return _pjrt_loop(nc, in_maps, iters)


def _build_baseline():
    if "base" in _CACHE:
        return _CACHE["base"]
    nc = bacc.Bacc("TRN2", target_bir_lowering=False, debug=False,
                   num_devices=NCORES)
    tin = nc.dram_tensor("bx", [128, 4], F32, kind="ExternalInput").ap()
    tout = nc.dram_tensor("bout", [128, 4], F32, kind="ExternalOutput").ap()
    with tile.TileContext(nc) as tc:
        with tc.tile_pool(name="p", bufs=1) as p:
            t = p.tile([128, 4], F32, name="bt", tag="bt")
            nc.sync.dma_start(out=t, in_=tin)
            nc.sync.dma_start(out=tout, in_=t)
    nc.compile()
    _CACHE["base"] = nc
    return nc


def _time_baseline(iters=20):
    nc = _build_baseline()
    in_maps = [{"bx": np.ones((128, 4), np.float32)} for _ in range(NCORES)]
    return _pjrt_loop(nc, in_maps, iters)


# revision 23
# speedup vs baseline: 1.1586x; 1.1586x over previous
"""Trainium2 Bass kernel for nn_L2GTraversal (leaf->level1->root point-cloud net).

Strategy (8 NeuronCores, data-parallel over leaves):
  - 64 leaves x 2048 points; core m owns leaves 8m..8m+7 (16384 points).
  - All activations kept TRANSPOSED (channels on partitions, points on the
    free dim) so every layer is lhsT=weight (stationary), rhs=activation^T,
    and the per-leaf max-pool is a free-dim reduce.
  - Algebraic fold: proj@We1[3:] with proj = relu1@Wp2 + bp2 is folded to
    relu1@(Wp2@We1[3:]) + const-bias, removing one 128x128 GEMM per point.
  - relu/max/bias commute: the last-layer relu+bias is applied after the
    per-leaf max-pool (on 512 values/leaf instead of 2048x512).
  - All matmul operands are bf16 (weights host-rounded, activations
    engine-rounded); bf16 stationary weights get 2x-faster LDWEIGHTS via
    fast-weight-load, and the PE runs 1 cycle/row (same as fp32r).
  - Relative coords (pts - leaf_center) are computed on the HOST and DMA'd
    in as relT (3, PTS) — computing them on GpSimd costs 29us/leaf and
    serializes the whole pipeline.
  - Chunk-pipelined emission (mm1 issued two chunks ahead) keeps Tensor/
    Scalar/Vector continuously busy so the HAM clock stays at 2.4 GHz.
    pe2 accumulates chunk PAIRS into (128,1024) two-bank PSUM tiles so the
    Vector engine drains them with half as many (fixed-overhead-dominated)
    reduce passes.
  - Level-1 aggregation is core-local (leaves 8m..8m+7 are exactly parent
    m's children).  The root needs the cross-core max of the per-parent
    relu(Wa1@[level1_m; relpos]) vectors: one tiny AllGather of (128,4)
    bf16 (~10us cheaper than AllReduce here), then each core reduces the
    gathered blocks locally and computes the root row.
  - A dummy AllGather issued at kernel start absorbs the collective
    stream-entry cost (~11.5us trigger latency) concurrently with the
    leaf pipeline, so the real tail AllGather triggers in ~1us.
  - Level1 is folded OUT of the root's critical path: g2 = relu(WaF^T m1
    + bg2) with WaF = Wa2@Wa1[:512] and bg2 = ba1 + ba2@Wa1[:512] +
    r2@Wa1[512:] host-precomputed (bg2 per-core).  The level1 output rows
    are computed during the collective window instead of before it.

Host side does only: index gathers, transposes/slicing for the chosen
sharding layout, the one-time weight fold, and output reassembly.
"""

import os

import numpy as np

import concourse.bass as bass  # noqa: F401
import concourse.mybir as mybir
import concourse.tile as tile
from concourse import bacc
from concourse.bass_utils import run_bass_kernel_spmd

NCORES = 8
L, K, C = 64, 2048, 32
LPC = L // NCORES            # leaves per core
PTS = LPC * K                # points per core
D_PROJ, D_HID, D = 128, 256, 512
CH = 512                     # point-chunk (matmul free dim)
CPL = K // CH                # chunks per leaf
F32 = mybir.dt.float32
BF16 = mybir.dt.bfloat16


def _bf16(a):
    import ml_dtypes
    return np.ascontiguousarray(np.asarray(a, np.float32).astype(
        ml_dtypes.bfloat16))


def _emit(tc, tin, tout):
    nc = tc.nc
    import contextlib

    ctx = contextlib.ExitStack()
    with ctx:
        const = ctx.enter_context(tc.tile_pool(name="const", bufs=1))
        act = ctx.enter_context(tc.tile_pool(name="act", bufs=1))
        red = ctx.enter_context(tc.tile_pool(name="red", bufs=1))
        agg = ctx.enter_context(tc.tile_pool(name="agg", bufs=1))
        psp = ctx.enter_context(tc.tile_pool(name="psum", bufs=1, space="PSUM"))
        dram = ctx.enter_context(tc.tile_pool(name="dram", bufs=1, space="DRAM"))

        def cload(name, shape, dt=F32):
            t = const.tile(list(shape), dt, name=name, tag=name)
            nc.sync.dma_start(out=t, in_=tin[name][:, :])
            return t

        RELU = mybir.ActivationFunctionType.Relu

        # ---- DMA priority order: first weights + leaf-0 data the pipeline
        # start needs, then the rest of the leaves, aggregation weights last --
        wp1 = cload("wp1", (32, 128), BF16)
        bp1 = cload("bp1", (128, 1))

        pfT, relT = {}, {}

        def load_leaf(l, part=None):
            # part: (tag_suffix, lo, hi) chunk range, else whole leaf
            lo, hi = (0, K) if part is None else part
            sfx = "" if part is None else f"_{lo}"
            t = const.tile([32, hi - lo], BF16, name=f"pfT{l}{sfx}",
                           tag=f"pfT{l}{sfx}")
            nc.sync.dma_start(out=t, in_=tin["featsT"][:, l * K + lo:l * K + hi])
            r = const.tile([3, hi - lo], BF16, name=f"relT{l}{sfx}",
                           tag=f"relT{l}{sfx}")
            nc.sync.dma_start(out=r, in_=tin["relT"][:, l * K + lo:l * K + hi])
            for c in range((hi - lo) // CH):
                pfT[l * CPL + lo // CH + c] = (t, c)
                relT[l * CPL + lo // CH + c] = (r, c)

        load_leaf(0, (0, 2 * CH))
        w2e = cload("w2e", (128, 256), BF16)
        we1a = cload("we1a", (3, 256), BF16)
        be1f = cload("be1f", (128, 2))
        we2 = []
        for kt in range(2):
            t = const.tile([128, 512], BF16, name=f"we2_{kt}", tag=f"we2_{kt}")
            nc.sync.dma_start(out=t, in_=tin["we2"][kt * 128:(kt + 1) * 128, :])
            we2.append(t)
        load_leaf(0, (2 * CH, K))
        be2c = cload("be2c", (128, 4))
        for l in range(1, LPC):
            load_leaf(l)

        def pf_sl(q):
            t, c = pfT[q]
            return t[:, c * CH:(c + 1) * CH]

        def rel_sl(q):
            t, c = relT[q]
            return t[:, c * CH:(c + 1) * CH]

        # aggregation weights (needed only at the end)
        wa1 = []
        for kt in range(4):
            t = const.tile([128, 512], BF16, name=f"wa1_{kt}", tag=f"wa1_{kt}")
            nc.sync.dma_start(out=t, in_=tin["wa1"][kt * 128:(kt + 1) * 128, :])
            wa1.append(t)
        wa1r = cload("wa1r", (3, 512), BF16)
        ba1c = cload("ba1c", (128, 4))
        wa2 = []
        for kt in range(4):
            t = const.tile([128, 512], BF16, name=f"wa2_{kt}", tag=f"wa2_{kt}")
            nc.sync.dma_start(out=t, in_=tin["wa2"][kt * 128:(kt + 1) * 128, :])
            wa2.append(t)
        ba2c = cload("ba2c", (128, 4))
        ba2r = cload("ba2r", (1, 512))
        relcT = cload("relcT", (3, LPC), BF16)
        waf = []
        for kt in range(4):
            t = const.tile([128, 512], BF16, name=f"waf_{kt}", tag=f"waf_{kt}")
            nc.sync.dma_start(out=t, in_=tin["waf"][kt * 128:(kt + 1) * 128, :])
            waf.append(t)
        bg2c = cload("bg2c", (128, 4))

        # leaf features (relu+bias applied), one column per leaf; bf16 so
        # they can be the moving operand of the bf16 aggregation matmuls
        lfv = [const.tile([128, LPC], BF16, name=f"lfv{o2}", tag=f"lfv{o2}")
               for o2 in range(4)]

        # warmup collective: pays the collective-stream entry costs while
        # the leaf pipeline runs, so the real AllGather at the tail doesn't
        if os.environ.get("KSKIP_CC") != "1":
            wsb = agg.tile([128, 4], BF16, name="wsb", tag="wsb")
            nc.vector.memset(wsb, 0.0)
            win = dram.tile([128, 4], BF16, name="wrmin", tag="wrmin")
            wout = dram.tile([NCORES, 128, 4], BF16, name="wrmout",
                             tag="wrmout")
            nc.gpsimd.dma_start(out=win, in_=wsb)
            nc.gpsimd.collective_compute(
                "AllGather",
                mybir.AluOpType.bypass,
                replica_groups=[list(range(NCORES))],
                ins=[win.opt()],
                outs=[wout.opt()],
            )

        relu1 = {}   # chunk -> (128, CH) sbuf tile
        hTd = {}     # pair -> [ot0, ot1] (128, 2*CH) bf16 tiles (chunk pair)
        mx = {}      # (leaf, o2) -> (128, 2) per-pair max columns

        def emit_mm1(q):
            l, c = q // CPL, q % CPL
            ps1 = psp.tile([128, CH], F32, name=f"ps1_q{q}", tag="ps1", bufs=2)
            nc.tensor.matmul(ps1, wp1, pf_sl(q), start=True, stop=True)
            t = act.tile([128, CH], BF16, name=f"relu1_q{q}", tag="relu1",
                         bufs=4)
            nc.scalar.activation(t, ps1, RELU, bias=bp1[:, 0:1])
            relu1[q] = t

        def emit_pe1(q):
            l, c = q // CPL, q % CPL
            if c % 2 == 0:
                hTd[q // 2] = [
                    act.tile([128, 2 * CH], BF16, name=f"hTd_p{q // 2}o{ot}",
                             tag=f"hTd{ot}", bufs=3)
                    for ot in range(2)]
            half = slice((c % 2) * CH, (c % 2 + 1) * CH)
            for ot in range(2):
                sl = slice(ot * 128, (ot + 1) * 128)
                pe1 = psp.tile([128, CH], F32, name=f"pe1_q{q}o{ot}",
                               tag="pe1", bufs=2)
                nc.tensor.matmul(pe1, w2e[:, sl], relu1[q],
                                 start=True, stop=False)
                nc.tensor.matmul(pe1, we1a[:, sl], rel_sl(q),
                                 start=False, stop=True)
                nc.scalar.activation(hTd[q // 2][ot][:, half], pe1, RELU,
                                     bias=be1f[:, ot:ot + 1])
            del relu1[q]

        def emit_rest(q):
            l, c = q // CPL, q % CPL
            emit_pe1(q)
            if c % 2 == 0:
                return
            # chunk pair (q-1, q): two accumulating 1024-free bf16 GEMMs per
            # o2 into a (128,1024) 2-bank psum tile, one 1024-wide reduce
            hd = hTd[q // 2]
            for o2 in range(4):
                sl = slice(o2 * 128, (o2 + 1) * 128)
                pe2 = psp.tile([128, 2 * CH], F32, name=f"pe2_q{q}o{o2}",
                               tag="pe2", bufs=2)
                for h in range(2):
                    half = slice(h * CH, (h + 1) * CH)
                    nc.tensor.matmul(pe2[:, half], we2[0][:, sl],
                                     hd[0][:, half], start=True, stop=False)
                    nc.tensor.matmul(pe2[:, half], we2[1][:, sl],
                                     hd[1][:, half], start=False, stop=True)
                if c == 1:
                    mx[(l, o2)] = red.tile([128, 2], F32, name=f"mx_l{l}o{o2}",
                                           tag=f"mx{o2}", bufs=2)
                nc.vector.reduce_max(out=mx[(l, o2)][:, c // 2:c // 2 + 1],
                                     in_=pe2, axis=mybir.AxisListType.X)
            del hTd[q // 2]
            if c == CPL - 1:
                for o2 in range(4):
                    lfm = red.tile([128, 1], F32, name=f"lfm_l{l}o{o2}",
                                   tag=f"lfm{o2}", bufs=2)
                    nc.vector.reduce_max(out=lfm, in_=mx.pop((l, o2)),
                                         axis=mybir.AxisListType.X)
                    nc.scalar.activation(lfv[o2][:, l:l + 1], lfm, RELU,
                                         bias=be2c[:, o2:o2 + 1])

        # ---- software-pipelined emission of the per-point encoder ----
        NQ = LPC * CPL
        emit_mm1(0)
        emit_mm1(1)
        for q in range(NQ):
            if q + 2 < NQ:
                emit_mm1(q + 2)
            emit_rest(q)

        # ---- leaf features out (convert to f32 for the output DMA) ----
        for o2 in range(4):
            cvt = agg.tile([128, LPC], F32, name=f"lfvf{o2}", tag=f"lfvf{o2}")
            nc.vector.tensor_copy(cvt, lfv[o2])
            nc.sync.dma_start(out=tout[o2 * 128:(o2 + 1) * 128, 0:LPC],
                              in_=cvt)

        # ---- level 1: g1 = relu(Wa1^T [lfv; relc] + ba1); max; @Wa2 + ba2 ----
        m1 = []
        for o2 in range(4):
            sl = slice(o2 * 128, (o2 + 1) * 128)
            psA = psp.tile([128, LPC], F32, name=f"psA{o2}", tag="pe2", bufs=2)
            for kt in range(4):
                nc.tensor.matmul(psA, wa1[kt][:, sl], lfv[kt],
                                 start=(kt == 0), stop=False)
            nc.tensor.matmul(psA, wa1r[:, sl], relcT, start=False, stop=True)
            g1 = agg.tile([128, LPC], BF16, name=f"g1_{o2}", tag=f"g1_{o2}")
            nc.scalar.activation(g1, psA, RELU, bias=ba1c[:, o2:o2 + 1])
            m = agg.tile([128, 1], BF16, name=f"m1_{o2}", tag=f"m1_{o2}")
            nc.vector.reduce_max(out=m, in_=g1, axis=mybir.AxisListType.X)
            m1.append(m)

        # ---- root pre-activation, with level1 folded out of the critical
        # path: Wa1^T(Wa2^T m1 + ba2) + Wa1r^T r2 + ba1 = WaF^T m1 + bg2
        # (WaF = Wa2 @ Wa1[:512] and bg2 are host-precomputed, bg2 per-core)
        g2 = agg.tile([128, 4], BF16, name="g2", tag="g2")
        for o2 in range(4):
            sl = slice(o2 * 128, (o2 + 1) * 128)
            psR = psp.tile([128, 1], F32, name=f"psR{o2}", tag="ps1", bufs=2)
            for kt in range(4):
                nc.tensor.matmul(psR, waf[kt][:, sl], m1[kt],
                                 start=(kt == 0), stop=(kt == 3))
            nc.scalar.activation(g2[:, o2:o2 + 1], psR, RELU,
                                 bias=bg2c[:, o2:o2 + 1])

        gm = agg.tile([128, 4], BF16, name="gm", tag="gm")
        if os.environ.get("KSKIP_CC") == "1":
            nc.vector.tensor_copy(gm, g2)
        else:
            crin = dram.tile([128, 4], BF16, name="crin", tag="crin")
            crout = dram.tile([NCORES, 128, 4], BF16, name="crout",
                              tag="crout")
            nc.sync.dma_start(out=crin, in_=g2)
            nc.gpsimd.collective_compute(
                "AllGather",
                mybir.AluOpType.bypass,
                replica_groups=[list(range(NCORES))],
                ins=[crin.opt()],
                outs=[crout.opt()],
            )
            gall = agg.tile([128, NCORES, 4], BF16, name="gall", tag="gall")
            nc.sync.dma_start(out=gall,
                               in_=crout[:, :, :].transpose([1, 0, 2]))

        # level1 output rows — computed during the collective window
        for o2 in range(4):
            sl = slice(o2 * 128, (o2 + 1) * 128)
            psL = psp.tile([128, 1], F32, name=f"psL{o2}", tag="ps1", bufs=2)
            for kt in range(4):
                nc.tensor.matmul(psL, wa2[kt][:, sl], m1[kt],
                                 start=(kt == 0), stop=(kt == 3))
            vf = agg.tile([128, 1], F32, name=f"lvl1f_{o2}", tag=f"lvl1f_{o2}")
            nc.scalar.add(vf, psL, ba2c[:, o2:o2 + 1])
            nc.sync.dma_start(out=tout[sl, LPC:LPC + 1], in_=vf)
            gall = gall.flatten_outer_dims() if False else gall
            # per-channel max over the 8 gathered blocks (stride-4 views)
            for c2 in range(4):
                nc.vector.reduce_max(out=gm[:, c2:c2 + 1],
                                     in_=gall[:, :, c2],
                                     axis=mybir.AxisListType.X)

        # root row as a single (1,512) vector: gm columns are the stationary
        # operand, Wa2 streams — 4 wide matmuls instead of 16 tiny ones
        psR2 = psp.tile([1, 512], F32, name="psR2", tag="ps1", bufs=2)
        for kt in range(4):
            nc.tensor.matmul(psR2, gm[:, kt:kt + 1], wa2[kt],
                             start=(kt == 0), stop=(kt == 3))
        rv = agg.tile([1, 512], F32, name="rootv", tag="rootv")
        nc.vector.tensor_tensor(out=rv, in0=psR2, in1=ba2r,
                                op=mybir.AluOpType.add)
        nc.sync.dma_start(out=tout[:, LPC + 1:LPC + 2], in_=rv)


_CACHE = {}


def _build():
    if "nc" in _CACHE:
        return _CACHE["nc"]
    nc = bacc.Bacc("TRN2", target_bir_lowering=False, debug=False,
                   num_devices=NCORES)
    shapes = {
        "featsT": ((32, PTS), BF16), "relT": ((3, PTS), BF16),
        "relcT": ((3, LPC), BF16), "waf": ((512, 512), BF16),
        "bg2c": ((128, 4), F32),
        "wp1": ((32, 128), BF16), "bp1": ((128, 1), F32),
        "w2e": ((128, 256), BF16), "we1a": ((3, 256), BF16),
        "be1f": ((128, 2), F32), "we2": ((256, 512), BF16),
        "be2c": ((128, 4), F32), "wa1": ((512, 512), BF16),
        "wa1r": ((3, 512), BF16), "ba1c": ((128, 4), F32),
        "wa2": ((512, 512), BF16), "ba2c": ((128, 4), F32),
        "ba2r": ((1, 512), F32),
    }
    tin = {name: nc.dram_tensor(name, list(shape), dt,
                                kind="ExternalInput").ap()
           for name, (shape, dt) in shapes.items()}
    tout = nc.dram_tensor("out", [512, LPC + 2], F32, kind="ExternalOutput").ap()
    with tile.TileContext(nc) as tc:
        _emit(tc, tin, tout)
    nc.compile()
    _CACHE["nc"] = nc
    return nc


def _prep_in_maps(inputs):
    f32 = np.float32
    coords = np.asarray(inputs["coords"], f32)
    feats = np.asarray(inputs["feats"], f32)
    leaf_indices = np.asarray(inputs["leaf_indices"])
    leaf_center_idx = np.asarray(inputs["leaf_center_idx"])
    l1_center_idx = np.asarray(inputs["l1_center_idx"])
    root_center_idx = int(np.asarray(inputs["root_center_idx"]))

    pts = coords[leaf_indices]            # (L, K, 3)
    pf = feats[leaf_indices]              # (L, K, C)
    centers = coords[leaf_center_idx]     # (L, 3)
    pp = coords[l1_center_idx]            # (B1, 3)
    rootc = coords[root_center_idx]       # (3,)

    Wp1 = np.asarray(inputs["Wp1"], f32)
    bp1 = np.asarray(inputs["bp1"], f32)
    Wp2 = np.asarray(inputs["Wp2"], f32)
    bp2 = np.asarray(inputs["bp2"], f32)
    We1 = np.asarray(inputs["We1"], f32)
    be1 = np.asarray(inputs["be1"], f32)
    We2 = np.asarray(inputs["We2"], f32)
    be2 = np.asarray(inputs["be2"], f32)
    Wa1 = np.asarray(inputs["Wa1"], f32)
    ba1 = np.asarray(inputs["ba1"], f32)
    Wa2 = np.asarray(inputs["Wa2"], f32)
    ba2 = np.asarray(inputs["ba2"], f32)

    # fold proj's second linear into the encoder first layer (fp64 for safety)
    We1a = np.ascontiguousarray(We1[0:3])                       # (3, 256)
    We1b = We1[3:131]                                           # (128, 256)
    W2e = (Wp2.astype(np.float64) @ We1b.astype(np.float64)).astype(f32)
    be1f = (be1.astype(np.float64)
            + bp2.astype(np.float64) @ We1b.astype(np.float64)).astype(f32)

    common = {
        "wp1": _bf16(Wp1),
        "bp1": np.ascontiguousarray(bp1.reshape(128, 1)),
        "w2e": _bf16(W2e),
        "we1a": _bf16(We1a),
        "be1f": np.ascontiguousarray(be1f.reshape(2, 128).T),
        "we2": _bf16(We2),
        "be2c": np.ascontiguousarray(be2.reshape(4, 128).T),
        "wa1": _bf16(Wa1[0:512]),
        "wa1r": _bf16(Wa1[512:515]),
        "ba1c": np.ascontiguousarray(ba1.reshape(4, 128).T),
        "wa2": _bf16(Wa2),
        "ba2c": np.ascontiguousarray(ba2.reshape(4, 128).T),
        "ba2r": np.ascontiguousarray(ba2.reshape(1, 512)),
        "waf": _bf16(Wa2.astype(np.float64) @ Wa1[0:512].astype(np.float64)),
    }

    in_maps = []
    for m in range(NCORES):
        sl = slice(m * LPC, (m + 1) * LPC)
        im = dict(common)
        im["featsT"] = _bf16(pf[sl].reshape(PTS, C).T)          # (32, PTS)
        rel = pts[sl] - centers[sl][:, None, :]                 # (LPC, K, 3)
        im["relT"] = _bf16(rel.reshape(PTS, 3).T)               # (3, PTS)
        im["relcT"] = _bf16((centers[sl] - pp[m]).T)
        r2 = (pp[m] - rootc).astype(np.float64)
        bg2 = (ba1.astype(np.float64)
               + ba2.astype(np.float64) @ Wa1[0:512].astype(np.float64)
               + r2 @ Wa1[512:515].astype(np.float64)).astype(f32)
        im["bg2c"] = np.ascontiguousarray(bg2.reshape(4, 128).T)
        in_maps.append(im)
    return in_maps


def _run(inputs, **kwargs):
    nc = _build()
    in_maps = _prep_in_maps(inputs)
    res = run_bass_kernel_spmd(nc, in_maps, core_ids=list(range(NCORES)),
                               **kwargs)
    out = np.empty((1 + NCORES + L, D), np.float32)
    out[0] = res.results[0]["out"][:, LPC + 1]
    for m in range(NCORES):
        out[1 + m] = res.results[m]["out"][:, LPC]
        out[1 + NCORES + m * LPC:1 + NCORES + (m + 1) * LPC] = \
            res.results[m]["out"][:, 0:LPC].T
    return out, res


def kernel(**inputs) -> np.ndarray:
    out, _ = _run(inputs)
    return out


# ---------------------------------------------------------------------------
# dev-only timing helpers (not used by kernel()); safe to keep — they only
# run when called explicitly from test.py.
# ---------------------------------------------------------------------------

def _pjrt_loop(nc, in_maps, iters):
    import time

    import jax
    from jax.experimental.shard_map import shard_map
    from jax.sharding import Mesh, NamedSharding, PartitionSpec

    from concourse.bass2jax import (_bass_exec_p, install_neuronx_cc_hook,
                                    partition_id_tensor)

    install_neuronx_cc_hook()
    pname = nc.partition_id_tensor.name if nc.partition_id_tensor else None
    in_names, out_names, out_avals, zero_outs = [], [], [], []
    for alloc in nc.m.functions[0].allocations:
        if not isinstance(alloc, mybir.MemoryLocationSet):
            continue
        name = alloc.memorylocations[0].name
        if alloc.kind == "ExternalInput":
            if name != pname:
                in_names.append(name)
        elif alloc.kind == "ExternalOutput":
            out_names.append(name)
            shape = tuple(alloc.tensor_shape)
            dtype = mybir.dt.np(alloc.dtype)
            out_avals.append(jax.core.ShapedArray(shape, dtype))
            zero_outs.append(np.zeros(shape, dtype))
    n_params = len(in_names)
    all_names = in_names + out_names
    if pname is not None:
        all_names = all_names + [pname]

    def _body(*args):
        operands = list(args)
        if pname is not None:
            operands.append(partition_id_tensor())
        outs = _bass_exec_p.bind(
            *operands, out_avals=tuple(out_avals), in_names=tuple(all_names),
            out_names=tuple(out_names), lowering_input_output_aliases=(),
            sim_require_finite=True, sim_require_nnan=True, nc=nc)
        return tuple(outs)

    ncores = len(in_maps)
    devices = jax.devices()[:ncores]
    mesh = Mesh(np.asarray(devices), ("core",))
    spec = PartitionSpec("core")
    donate = tuple(range(n_params, n_params + len(out_names)))
    fn = jax.jit(
        shard_map(_body, mesh=mesh,
                  in_specs=(spec,) * (n_params + len(out_names)),
                  out_specs=(spec,) * len(out_names), check_rep=False),
        donate_argnums=donate, keep_unused=True)
    sh = NamedSharding(mesh, spec)
    ins = [jax.device_put(
        np.concatenate([np.asarray(m[n]) for m in in_maps], axis=0), sh)
        for n in in_names]
    zs_proto = [np.zeros((ncores * z.shape[0], *z.shape[1:]), z.dtype)
                for z in zero_outs]
    outs = fn(*ins, *[jax.device_put(z, sh) for z in zs_proto])
    jax.block_until_ready(outs)
    times = []
    for _ in range(iters):
        zs = [jax.device_put(z, sh) for z in zs_proto]
        jax.block_until_ready(zs)
        t0 = time.perf_counter()
        outs = fn(*ins, *zs)
        jax.block_until_ready(outs)
        times.append(time.perf_counter() - t0)
    return times


def _time_hw(inputs, iters=20):
    nc = _build()
    in_maps = _prep_in_maps(inputs)
    return _pjrt_loop(nc, in_maps, iters)


def _build_baseline():
    if "base" in _CACHE:
        return _CACHE["base"]
    nc = bacc.Bacc("TRN2", target_bir_lowering=False, debug=False,
                   num_devices=NCORES)
    tin = nc.dram_tensor("bx", [128, 4], F32, kind="ExternalInput").ap()
    tout = nc.dram_tensor("bout", [128, 4], F32, kind="ExternalOutput").ap()
    with tile.TileContext(nc) as tc:
        with tc.tile_pool(name="p", bufs=1) as p:
            t = p.tile([128, 4], F32, name="bt", tag="bt")
            nc.sync.dma_start(out=t, in_=tin)
            nc.sync.dma_start(out=tout, in_=t)
    nc.compile()
    _CACHE["base"] = nc
    return nc


def _time_baseline(iters=20):
    nc = _build_baseline()
    in_maps = [{"bx": np.ones((128, 4), np.float32)} for _ in range(NCORES)]
    return _pjrt_loop(nc, in_maps, iters)


# revision 24
# speedup vs baseline: 1.2431x; 1.0730x over previous
"""Trainium2 Bass kernel for nn_L2GTraversal (leaf->level1->root point-cloud net).

Strategy (8 NeuronCores, data-parallel over leaves):
  - 64 leaves x 2048 points; core m owns leaves 8m..8m+7 (16384 points).
  - All activations kept TRANSPOSED (channels on partitions, points on the
    free dim) so every layer is lhsT=weight (stationary), rhs=activation^T,
    and the per-leaf max-pool is a free-dim reduce.
  - Algebraic fold: proj@We1[3:] with proj = relu1@Wp2 + bp2 is folded to
    relu1@(Wp2@We1[3:]) + const-bias, removing one 128x128 GEMM per point.
  - relu/max/bias commute: the last-layer relu+bias is applied after the
    per-leaf max-pool (on 512 values/leaf instead of 2048x512).
  - All matmul operands are bf16 (weights host-rounded, activations
    engine-rounded); bf16 stationary weights get 2x-faster LDWEIGHTS via
    fast-weight-load, and the PE runs 1 cycle/row (same as fp32r).
  - Relative coords (pts - leaf_center) are computed on the HOST and DMA'd
    in as relT (3, PTS) — computing them on GpSimd costs 29us/leaf and
    serializes the whole pipeline.
  - Chunk-pipelined emission (mm1 issued two chunks ahead) keeps Tensor/
    Scalar/Vector continuously busy so the HAM clock stays at 2.4 GHz.
    pe2 accumulates chunk PAIRS into (128,1024) two-bank PSUM tiles so the
    Vector engine drains them with half as many (fixed-overhead-dominated)
    reduce passes.
  - Level-1 aggregation is core-local (leaves 8m..8m+7 are exactly parent
    m's children).  The root needs the cross-core max of the per-parent
    relu(Wa1@[level1_m; relpos]) vectors: one tiny AllGather of (128,4)
    bf16 (~10us cheaper than AllReduce here), then each core reduces the
    gathered blocks locally and computes the root row.
  - A dummy AllGather issued at kernel start absorbs the collective
    stream-entry cost (~11.5us trigger latency) concurrently with the
    leaf pipeline, so the real tail AllGather triggers in ~1us.
  - Level1 is folded OUT of the root's critical path: g2 = relu(WaF^T m1
    + bg2) with WaF = Wa2@Wa1[:512] and bg2 = ba1 + ba2@Wa1[:512] +
    r2@Wa1[512:] host-precomputed (bg2 per-core).  The level1 output rows
    are computed during the collective window instead of before it.

Host side does only: index gathers, transposes/slicing for the chosen
sharding layout, the one-time weight fold, and output reassembly.
"""

import os

import numpy as np

import concourse.bass as bass  # noqa: F401
import concourse.mybir as mybir
import concourse.tile as tile
from concourse import bacc
from concourse.bass_utils import run_bass_kernel_spmd

NCORES = 8
L, K, C = 64, 2048, 32
LPC = L // NCORES            # leaves per core
PTS = LPC * K                # points per core
D_PROJ, D_HID, D = 128, 256, 512
CH = 512                     # point-chunk (matmul free dim)
CPL = K // CH                # chunks per leaf
F32 = mybir.dt.float32
BF16 = mybir.dt.bfloat16
FP8 = mybir.dt.float8e4


def _bf16(a):
    import ml_dtypes
    return np.ascontiguousarray(np.asarray(a, np.float32).astype(
        ml_dtypes.bfloat16))


def _fp8(a):
    import ml_dtypes
    return np.ascontiguousarray(np.asarray(a, np.float32).astype(
        ml_dtypes.float8_e4m3))


def _emit(tc, tin, tout):
    nc = tc.nc
    import contextlib

    ctx = contextlib.ExitStack()
    with ctx:
        const = ctx.enter_context(tc.tile_pool(name="const", bufs=1))
        act = ctx.enter_context(tc.tile_pool(name="act", bufs=1))
        red = ctx.enter_context(tc.tile_pool(name="red", bufs=1))
        agg = ctx.enter_context(tc.tile_pool(name="agg", bufs=1))
        psp = ctx.enter_context(tc.tile_pool(name="psum", bufs=1, space="PSUM"))
        dram = ctx.enter_context(tc.tile_pool(name="dram", bufs=1, space="DRAM"))

        def cload(name, shape, dt=F32):
            t = const.tile(list(shape), dt, name=name, tag=name)
            nc.sync.dma_start(out=t, in_=tin[name][:, :])
            return t

        RELU = mybir.ActivationFunctionType.Relu

        # ---- DMA priority order: first weights + leaf-0 data the pipeline
        # start needs, then the rest of the leaves, aggregation weights last --
        wp1 = cload("wp1", (32, 128), BF16)
        bp1 = cload("bp1", (128, 1))

        pfT, relT = {}, {}

        def load_leaf(l, part=None):
            # part: (tag_suffix, lo, hi) chunk range, else whole leaf
            lo, hi = (0, K) if part is None else part
            sfx = "" if part is None else f"_{lo}"
            t = const.tile([32, hi - lo], BF16, name=f"pfT{l}{sfx}",
                           tag=f"pfT{l}{sfx}")
            nc.sync.dma_start(out=t, in_=tin["featsT"][:, l * K + lo:l * K + hi])
            r = const.tile([3, hi - lo], BF16, name=f"relT{l}{sfx}",
                           tag=f"relT{l}{sfx}")
            nc.sync.dma_start(out=r, in_=tin["relT"][:, l * K + lo:l * K + hi])
            for c in range((hi - lo) // CH):
                pfT[l * CPL + lo // CH + c] = (t, c)
                relT[l * CPL + lo // CH + c] = (r, c)

        load_leaf(0, (0, 2 * CH))
        w2e = cload("w2e", (128, 256), BF16)
        we1a = cload("we1a", (3, 256), BF16)
        be1f = cload("be1f", (128, 2))
        we2 = []
        for kt in range(2):
            t = const.tile([128, 512], FP8, name=f"we2_{kt}", tag=f"we2_{kt}")
            nc.sync.dma_start(out=t, in_=tin["we2"][kt * 128:(kt + 1) * 128, :])
            we2.append(t)
        load_leaf(0, (2 * CH, K))
        be2c = cload("be2c", (128, 4))
        for l in range(1, LPC):
            load_leaf(l)

        def pf_sl(q):
            t, c = pfT[q]
            return t[:, c * CH:(c + 1) * CH]

        def rel_sl(q):
            t, c = relT[q]
            return t[:, c * CH:(c + 1) * CH]

        # aggregation weights (needed only at the end)
        wa1 = []
        for kt in range(4):
            t = const.tile([128, 512], BF16, name=f"wa1_{kt}", tag=f"wa1_{kt}")
            nc.sync.dma_start(out=t, in_=tin["wa1"][kt * 128:(kt + 1) * 128, :])
            wa1.append(t)
        wa1r = cload("wa1r", (3, 512), BF16)
        ba1c = cload("ba1c", (128, 4))
        wa2 = []
        for kt in range(4):
            t = const.tile([128, 512], BF16, name=f"wa2_{kt}", tag=f"wa2_{kt}")
            nc.sync.dma_start(out=t, in_=tin["wa2"][kt * 128:(kt + 1) * 128, :])
            wa2.append(t)
        ba2c = cload("ba2c", (128, 4))
        ba2r = cload("ba2r", (1, 512))
        relcT = cload("relcT", (3, LPC), BF16)
        waf = []
        for kt in range(4):
            t = const.tile([128, 512], BF16, name=f"waf_{kt}", tag=f"waf_{kt}")
            nc.sync.dma_start(out=t, in_=tin["waf"][kt * 128:(kt + 1) * 128, :])
            waf.append(t)
        bg2c = cload("bg2c", (128, 4))

        # leaf features (relu+bias applied), one column per leaf; bf16 so
        # they can be the moving operand of the bf16 aggregation matmuls
        lfv = [const.tile([128, LPC], BF16, name=f"lfv{o2}", tag=f"lfv{o2}")
               for o2 in range(4)]

        # warmup collective: pays the collective-stream entry costs while
        # the leaf pipeline runs, so the real AllGather at the tail doesn't
        if os.environ.get("KSKIP_CC") != "1":
            wsb = agg.tile([128, 4], BF16, name="wsb", tag="wsb")
            nc.vector.memset(wsb, 0.0)
            win = dram.tile([128, 4], BF16, name="wrmin", tag="wrmin")
            wout = dram.tile([NCORES, 128, 4], BF16, name="wrmout",
                             tag="wrmout")
            nc.gpsimd.dma_start(out=win, in_=wsb)
            nc.gpsimd.collective_compute(
                "AllGather",
                mybir.AluOpType.bypass,
                replica_groups=[list(range(NCORES))],
                ins=[win.opt()],
                outs=[wout.opt()],
            )

        relu1 = {}   # chunk -> (128, CH) sbuf tile
        hTd = {}     # pair -> [ot0, ot1] (128, 2*CH) bf16 tiles (chunk pair)
        mx = {}      # (leaf, o2) -> (128, 2) per-pair max columns

        def emit_mm1(q):
            l, c = q // CPL, q % CPL
            ps1 = psp.tile([128, CH], F32, name=f"ps1_q{q}", tag="ps1", bufs=2)
            nc.tensor.matmul(ps1, wp1, pf_sl(q), start=True, stop=True)
            t = act.tile([128, CH], BF16, name=f"relu1_q{q}", tag="relu1",
                         bufs=4)
            nc.scalar.activation(t, ps1, RELU, bias=bp1[:, 0:1])
            relu1[q] = t

        def emit_pe1(q):
            l, c = q // CPL, q % CPL
            if c % 2 == 0:
                hTd[q // 2] = [
                    act.tile([128, 2 * CH], BF16, name=f"hTd_p{q // 2}o{ot}",
                             tag=f"hTd{ot}", bufs=3)
                    for ot in range(2)]
            half = slice((c % 2) * CH, (c % 2 + 1) * CH)
            for ot in range(2):
                sl = slice(ot * 128, (ot + 1) * 128)
                pe1 = psp.tile([128, CH], F32, name=f"pe1_q{q}o{ot}",
                               tag="pe1", bufs=2)
                nc.tensor.matmul(pe1, w2e[:, sl], relu1[q],
                                 start=True, stop=False)
                nc.tensor.matmul(pe1, we1a[:, sl], rel_sl(q),
                                 start=False, stop=True)
                nc.scalar.activation(hTd[q // 2][ot][:, half], pe1, RELU,
                                     bias=be1f[:, ot:ot + 1])
            del relu1[q]

        def emit_rest(q):
            l, c = q // CPL, q % CPL
            emit_pe1(q)
            if c % 2 == 0:
                return
            # chunk pair (q-1, q): two accumulating 1024-free bf16 GEMMs per
            # o2 into a (128,1024) 2-bank psum tile, one 1024-wide reduce
            hd = hTd[q // 2]
            for o2 in range(4):
                sl = slice(o2 * 128, (o2 + 1) * 128)
                pe2 = psp.tile([128, 2 * CH], F32, name=f"pe2_q{q}o{o2}",
                               tag="pe2", bufs=2)
                for h in range(2):
                    half = slice(h * CH, (h + 1) * CH)
                    nc.tensor.matmul(pe2[:, half], we2[0][:, sl],
                                     hd[0][:, half], start=True, stop=False)
                    nc.tensor.matmul(pe2[:, half], we2[1][:, sl],
                                     hd[1][:, half], start=False, stop=True)
                if c == 1:
                    mx[(l, o2)] = red.tile([128, 2], F32, name=f"mx_l{l}o{o2}",
                                           tag=f"mx{o2}", bufs=2)
                nc.vector.reduce_max(out=mx[(l, o2)][:, c // 2:c // 2 + 1],
                                     in_=pe2, axis=mybir.AxisListType.X)
            del hTd[q // 2]
            if c == CPL - 1:
                for o2 in range(4):
                    lfm = red.tile([128, 1], F32, name=f"lfm_l{l}o{o2}",
                                   tag=f"lfm{o2}", bufs=2)
                    nc.vector.reduce_max(out=lfm, in_=mx.pop((l, o2)),
                                         axis=mybir.AxisListType.X)
                    nc.scalar.activation(lfv[o2][:, l:l + 1], lfm, RELU,
                                         bias=be2c[:, o2:o2 + 1])

        # ---- software-pipelined emission of the per-point encoder ----
        NQ = LPC * CPL
        emit_mm1(0)
        emit_mm1(1)
        for q in range(NQ):
            if q + 2 < NQ:
                emit_mm1(q + 2)
            emit_rest(q)

        # ---- leaf features out (convert to f32 for the output DMA) ----
        for o2 in range(4):
            cvt = agg.tile([128, LPC], F32, name=f"lfvf{o2}", tag=f"lfvf{o2}")
            nc.vector.tensor_copy(cvt, lfv[o2])
            nc.sync.dma_start(out=tout[o2 * 128:(o2 + 1) * 128, 0:LPC],
                              in_=cvt)

        # ---- level 1: g1 = relu(Wa1^T [lfv; relc] + ba1); max; @Wa2 + ba2 ----
        m1 = []
        for o2 in range(4):
            sl = slice(o2 * 128, (o2 + 1) * 128)
            psA = psp.tile([128, LPC], F32, name=f"psA{o2}", tag="pe2", bufs=2)
            for kt in range(4):
                nc.tensor.matmul(psA, wa1[kt][:, sl], lfv[kt],
                                 start=(kt == 0), stop=False)
            nc.tensor.matmul(psA, wa1r[:, sl], relcT, start=False, stop=True)
            g1 = agg.tile([128, LPC], BF16, name=f"g1_{o2}", tag=f"g1_{o2}")
            nc.scalar.activation(g1, psA, RELU, bias=ba1c[:, o2:o2 + 1])
            m = agg.tile([128, 1], BF16, name=f"m1_{o2}", tag=f"m1_{o2}")
            nc.vector.reduce_max(out=m, in_=g1, axis=mybir.AxisListType.X)
            m1.append(m)

        # ---- root pre-activation, with level1 folded out of the critical
        # path: Wa1^T(Wa2^T m1 + ba2) + Wa1r^T r2 + ba1 = WaF^T m1 + bg2
        # (WaF = Wa2 @ Wa1[:512] and bg2 are host-precomputed, bg2 per-core)
        g2 = agg.tile([128, 4], BF16, name="g2", tag="g2")
        for o2 in range(4):
            sl = slice(o2 * 128, (o2 + 1) * 128)
            psR = psp.tile([128, 1], F32, name=f"psR{o2}", tag="ps1", bufs=2)
            for kt in range(4):
                nc.tensor.matmul(psR, waf[kt][:, sl], m1[kt],
                                 start=(kt == 0), stop=(kt == 3))
            nc.scalar.activation(g2[:, o2:o2 + 1], psR, RELU,
                                 bias=bg2c[:, o2:o2 + 1])

        gm = agg.tile([128, 4], BF16, name="gm", tag="gm")
        if os.environ.get("KSKIP_CC") == "1":
            nc.vector.tensor_copy(gm, g2)
        else:
            crin = dram.tile([128, 4], BF16, name="crin", tag="crin")
            crout = dram.tile([NCORES, 128, 4], BF16, name="crout",
                              tag="crout")
            nc.sync.dma_start(out=crin, in_=g2)
            nc.gpsimd.collective_compute(
                "AllGather",
                mybir.AluOpType.bypass,
                replica_groups=[list(range(NCORES))],
                ins=[crin.opt()],
                outs=[crout.opt()],
            )
            gall = agg.tile([128, NCORES, 4], BF16, name="gall", tag="gall")
            nc.sync.dma_start(out=gall,
                               in_=crout[:, :, :].transpose([1, 0, 2]))

        # level1 output rows — computed during the collective window
        for o2 in range(4):
            sl = slice(o2 * 128, (o2 + 1) * 128)
            psL = psp.tile([128, 1], F32, name=f"psL{o2}", tag="ps1", bufs=2)
            for kt in range(4):
                nc.tensor.matmul(psL, wa2[kt][:, sl], m1[kt],
                                 start=(kt == 0), stop=(kt == 3))
            vf = agg.tile([128, 1], F32, name=f"lvl1f_{o2}", tag=f"lvl1f_{o2}")
            nc.scalar.add(vf, psL, ba2c[:, o2:o2 + 1])
            nc.sync.dma_start(out=tout[sl, LPC:LPC + 1], in_=vf)
            gall = gall.flatten_outer_dims() if False else gall
            # per-channel max over the 8 gathered blocks (stride-4 views)
            for c2 in range(4):
                nc.vector.reduce_max(out=gm[:, c2:c2 + 1],
                                     in_=gall[:, :, c2],
                                     axis=mybir.AxisListType.X)

        # root row as a single (1,512) vector: gm columns are the stationary
        # operand, Wa2 streams — 4 wide matmuls instead of 16 tiny ones
        psR2 = psp.tile([1, 512], F32, name="psR2", tag="ps1", bufs=2)
        for kt in range(4):
            nc.tensor.matmul(psR2, gm[:, kt:kt + 1], wa2[kt],
                             start=(kt == 0), stop=(kt == 3))
        rv = agg.tile([1, 512], F32, name="rootv", tag="rootv")
        nc.vector.tensor_tensor(out=rv, in0=psR2, in1=ba2r,
                                op=mybir.AluOpType.add)
        nc.sync.dma_start(out=tout[:, LPC + 1:LPC + 2], in_=rv)


_CACHE = {}


def _build():
    if "nc" in _CACHE:
        return _CACHE["nc"]
    nc = bacc.Bacc("TRN2", target_bir_lowering=False, debug=False,
                   num_devices=NCORES)
    shapes = {
        "featsT": ((32, PTS), BF16), "relT": ((3, PTS), BF16),
        "relcT": ((3, LPC), BF16), "waf": ((512, 512), BF16),
        "bg2c": ((128, 4), F32),
        "wp1": ((32, 128), BF16), "bp1": ((128, 1), F32),
        "w2e": ((128, 256), BF16), "we1a": ((3, 256), BF16),
        "be1f": ((128, 2), F32), "we2": ((256, 512), FP8),
        "be2c": ((128, 4), F32), "wa1": ((512, 512), BF16),
        "wa1r": ((3, 512), BF16), "ba1c": ((128, 4), F32),
        "wa2": ((512, 512), BF16), "ba2c": ((128, 4), F32),
        "ba2r": ((1, 512), F32),
    }
    tin = {name: nc.dram_tensor(name, list(shape), dt,
                                kind="ExternalInput").ap()
           for name, (shape, dt) in shapes.items()}
    tout = nc.dram_tensor("out", [512, LPC + 2], F32, kind="ExternalOutput").ap()
    with tile.TileContext(nc) as tc:
        _emit(tc, tin, tout)
    nc.compile()
    _CACHE["nc"] = nc
    return nc


def _prep_in_maps(inputs):
    f32 = np.float32
    coords = np.asarray(inputs["coords"], f32)
    feats = np.asarray(inputs["feats"], f32)
    leaf_indices = np.asarray(inputs["leaf_indices"])
    leaf_center_idx = np.asarray(inputs["leaf_center_idx"])
    l1_center_idx = np.asarray(inputs["l1_center_idx"])
    root_center_idx = int(np.asarray(inputs["root_center_idx"]))

    pts = coords[leaf_indices]            # (L, K, 3)
    pf = feats[leaf_indices]              # (L, K, C)
    centers = coords[leaf_center_idx]     # (L, 3)
    pp = coords[l1_center_idx]            # (B1, 3)
    rootc = coords[root_center_idx]       # (3,)

    Wp1 = np.asarray(inputs["Wp1"], f32)
    bp1 = np.asarray(inputs["bp1"], f32)
    Wp2 = np.asarray(inputs["Wp2"], f32)
    bp2 = np.asarray(inputs["bp2"], f32)
    We1 = np.asarray(inputs["We1"], f32)
    be1 = np.asarray(inputs["be1"], f32)
    We2 = np.asarray(inputs["We2"], f32)
    be2 = np.asarray(inputs["be2"], f32)
    Wa1 = np.asarray(inputs["Wa1"], f32)
    ba1 = np.asarray(inputs["ba1"], f32)
    Wa2 = np.asarray(inputs["Wa2"], f32)
    ba2 = np.asarray(inputs["ba2"], f32)

    # fold proj's second linear into the encoder first layer (fp64 for safety)
    We1a = np.ascontiguousarray(We1[0:3])                       # (3, 256)
    We1b = We1[3:131]                                           # (128, 256)
    W2e = (Wp2.astype(np.float64) @ We1b.astype(np.float64)).astype(f32)
    be1f = (be1.astype(np.float64)
            + bp2.astype(np.float64) @ We1b.astype(np.float64)).astype(f32)

    common = {
        "wp1": _bf16(Wp1),
        "bp1": np.ascontiguousarray(bp1.reshape(128, 1)),
        "w2e": _bf16(W2e),
        "we1a": _bf16(We1a),
        "be1f": np.ascontiguousarray(be1f.reshape(2, 128).T),
        "we2": _fp8(We2),
        "be2c": np.ascontiguousarray(be2.reshape(4, 128).T),
        "wa1": _bf16(Wa1[0:512]),
        "wa1r": _bf16(Wa1[512:515]),
        "ba1c": np.ascontiguousarray(ba1.reshape(4, 128).T),
        "wa2": _bf16(Wa2),
        "ba2c": np.ascontiguousarray(ba2.reshape(4, 128).T),
        "ba2r": np.ascontiguousarray(ba2.reshape(1, 512)),
        "waf": _bf16(Wa2.astype(np.float64) @ Wa1[0:512].astype(np.float64)),
    }

    in_maps = []
    for m in range(NCORES):
        sl = slice(m * LPC, (m + 1) * LPC)
        im = dict(common)
        im["featsT"] = _bf16(pf[sl].reshape(PTS, C).T)          # (32, PTS)
        rel = pts[sl] - centers[sl][:, None, :]                 # (LPC, K, 3)
        im["relT"] = _bf16(rel.reshape(PTS, 3).T)               # (3, PTS)
        im["relcT"] = _bf16((centers[sl] - pp[m]).T)
        r2 = (pp[m] - rootc).astype(np.float64)
        bg2 = (ba1.astype(np.float64)
               + ba2.astype(np.float64) @ Wa1[0:512].astype(np.float64)
               + r2 @ Wa1[512:515].astype(np.float64)).astype(f32)
        im["bg2c"] = np.ascontiguousarray(bg2.reshape(4, 128).T)
        in_maps.append(im)
    return in_maps


def _run(inputs, **kwargs):
    nc = _build()
    in_maps = _prep_in_maps(inputs)
    res = run_bass_kernel_spmd(nc, in_maps, core_ids=list(range(NCORES)),
                               **kwargs)
    out = np.empty((1 + NCORES + L, D), np.float32)
    out[0] = res.results[0]["out"][:, LPC + 1]
    for m in range(NCORES):
        out[1 + m] = res.results[m]["out"][:, LPC]
        out[1 + NCORES + m * LPC:1 + NCORES + (m + 1) * LPC] = \
            res.results[m]["out"][:, 0:LPC].T
    return out, res


def kernel(**inputs) -> np.ndarray:
    out, _ = _run(inputs)
    return out


# ---------------------------------------------------------------------------
# dev-only timing helpers (not used by kernel()); safe to keep — they only
# run when called explicitly from test.py.
# ---------------------------------------------------------------------------

def _pjrt_loop(nc, in_maps, iters):
    import time

    import jax
    from jax.experimental.shard_map import shard_map
    from jax.sharding import Mesh, NamedSharding, PartitionSpec

    from concourse.bass2jax import (_bass_exec_p, install_neuronx_cc_hook,
                                    partition_id_tensor)

    install_neuronx_cc_hook()
    pname = nc.partition_id_tensor.name if nc.partition_id_tensor else None
    in_names, out_names, out_avals, zero_outs = [], [], [], []
    for alloc in nc.m.functions[0].allocations:
        if not isinstance(alloc, mybir.MemoryLocationSet):
            continue
        name = alloc.memorylocations[0].name
        if alloc.kind == "ExternalInput":
            if name != pname:
                in_names.append(name)
        elif alloc.kind == "ExternalOutput":
            out_names.append(name)
            shape = tuple(alloc.tensor_shape)
            dtype = mybir.dt.np(alloc.dtype)
            out_avals.append(jax.core.ShapedArray(shape, dtype))
            zero_outs.append(np.zeros(shape, dtype))
    n_params = len(in_names)
    all_names = in_names + out_names
    if pname is not None:
        all_names = all_names + [pname]

    def _body(*args):
        operands = list(args)
        if pname is not None:
            operands.append(partition_id_tensor())
        outs = _bass_exec_p.bind(
            *operands, out_avals=tuple(out_avals), in_names=tuple(all_names),
            out_names=tuple(out_names), lowering_input_output_aliases=(),
            sim_require_finite=True, sim_require_nnan=True, nc=nc)
        return tuple(outs)

    ncores = len(in_maps)
    devices = jax.devices()[:ncores]
    mesh = Mesh(np.asarray(devices), ("core",))
    spec = PartitionSpec("core")
    donate = tuple(range(n_params, n_params + len(out_names)))
    fn = jax.jit(
        shard_map(_body, mesh=mesh,
                  in_specs=(spec,) * (n_params + len(out_names)),
                  out_specs=(spec,) * len(out_names), check_rep=False),
        donate_argnums=donate, keep_unused=True)
    sh = NamedSharding(mesh, spec)
    ins = [jax.device_put(
        np.concatenate([np.asarray(m[n]) for m in in_maps], axis=0), sh)
        for n in in_names]
    zs_proto = [np.zeros((ncores * z.shape[0], *z.shape[1:]), z.dtype)
                for z in zero_outs]
    outs = fn(*ins, *[jax.device_put(z, sh) for z in zs_proto])
    jax.block_until_ready(outs)
    times = []
    for _ in range(iters):
        zs = [jax.device_put(z, sh) for z in zs_proto]
        jax.block_until_ready(zs)
        t0 = time.perf_counter()
        outs = fn(*ins, *zs)
        jax.block_until_ready(outs)
        times.append(time.perf_counter() - t0)
    return times


def _time_hw(inputs, iters=20):
    nc = _build()
    in_maps = _prep_in_maps(inputs)
    return _pjrt_loop(nc, in_maps, iters)


def _build_baseline():
    if "base" in _CACHE:
        return _CACHE["base"]
    nc = bacc.Bacc("TRN2", target_bir_lowering=False, debug=False,
                   num_devices=NCORES)
    tin = nc.dram_tensor("bx", [128, 4], F32, kind="ExternalInput").ap()
    tout = nc.dram_tensor("bout", [128, 4], F32, kind="ExternalOutput").ap()
    with tile.TileContext(nc) as tc:
        with tc.tile_pool(name="p", bufs=1) as p:
            t = p.tile([128, 4], F32, name="bt", tag="bt")
            nc.sync.dma_start(out=t, in_=tin)
            nc.sync.dma_start(out=tout, in_=t)
    nc.compile()
    _CACHE["base"] = nc
    return nc


def _time_baseline(iters=20):
    nc = _build_baseline()
    in_maps = [{"bx": np.ones((128, 4), np.float32)} for _ in range(NCORES)]
    return _pjrt_loop(nc, in_maps, iters)
